# revision 51
# baseline (speedup 1.0000x reference)
"""Trainium2 Bass kernel for nn_Attention_21835613733572 — v23 (j8, P2, fp8 k, dual-queue k split).

reference:
    score = einsum('bci,bcj->bij', k, q) / sqrt(L)       # (B, L, L)
    score = softmax(score, axis=0)                       # over the BATCH axis
    out   = einsum('bci,bij->bcj', v, score)             # (B, C, L)
with B, C, L = 16, 512, 1024 (f32 inputs/outputs).

Distribution: 8 j-slices of 128 columns, one per core; every core holds the
full k and v (the batch-axis softmax needs all 16 batches per (i,j), and
collectives on this fleet cost a fixed ~85us per NEFF — measured — so any
cross-core exchange loses). This is the baseline's sharding, rebuilt around
what the baseline trace showed:

 * P2 layout: MM1 uses stationary k[c,i_tile], moving q[c,j], so scores land
   as e[i_part, j_free] — exactly the lhsT layout MM2 needs. Removes all PE
   transpose matmuls (~14us of PE) the baseline spent.
 * ScalarE runs ONLY Exp (one activation table load, 1.28us per swap
   otherwise): MM1 psums are batch-quad packed ([128,4,128] = one full psum
   bank) so exp is 32 instructions instead of 128+ mixed exp/copy.
 * Denominator d[i,j] = sum_b e[b,i,j] via a 4-level strided pairwise fold
   on VectorE (4 instructions per i-tile, operands [128,8,128]...), not a
   15-add scalar tree.
 * MM2 psum evacuation on VectorE (no scalar table thrash), output DMA on
   the ACT hardware queue (the baseline put it on gpsimd's software DGE,
   which sustained only ~50 GB/s and added a ~30us tail).
 * k then v stream on the sync hardware queue (baseline showed a single
   HWDGE queue sustains ~385 GB/s; total per-core HBM ~420 GB/s).

Per-core input DMA 27.3MB (k fp8 8.4 + q 2.1 + v 16.8). Measured on this
fleet: 93.4us HW exec in the fleet's fast state, ~100-104 in its slow
state (runs are bimodal, +-5us; evaluate variants by min over >=3
samples — test.py runs the NEFF 3x per invocation for this), rel err
1.777e-2 (gate 2e-2; deterministic for the fixed-seed harness inputs).

Refinements over v9 (97.7 fast-state), all from NTFF trace forensics:
 * k4..k7 + q ride the scalar HWDGE queue; k0..k3 + v own the sync
   queue. The critical sync stream drops 25.2->21MB (~-8us stream end).
   The scalar queue is idle after q (~t18) and delivers k4-7 by ~t35,
   far ahead of MM1's need (~t43-55). k3 on scalar too is TOO MUCH (the
   early steal from Q1 outweighs; measured +8us), as is a 50/50 split
   of v (both queues drop to ~170 GB/s each — the per-core total is
   capped ~340-450 during compute and the k stream starves).
 * ALL k tiles ring-free (kpool bufs=8): a ring-gated k dma_start on
   the scalar engine would deadlock: its gate needs MM1 reads -> psum
   ring -> exps, and the exps sit BEHIND it in scalar program order.
 * out DMAs merged 4-batches-at-a-time on scalar: every dma_start draws
   a completion semaphore from a small pool (~8-10) shared across BOTH
   queues and waits for that semaphore's previous users to COMPLETE.
   With 16 per-batch outs, a late (MM2-gated) out completion gated the
   v8+ dma_start issues -> the k/v queue ran dry ~6us (visible as a
   cum-bytes plateau at t=60 in the v9 trace).
 * second-half prob muls hoisted ahead of the MM2 loop: MM2 chains no
   longer serialize on VectorE's cast(b-1) -> prob(b) program order.
 * ps1 bufs 4 (ps1+ps2 = all 8 PSUM banks); vpool 10, lpool 1, dpool 2
   pay the SBUF for kpool 8 (total ~207.5KB of ~208).

Experiments that regressed or tied (kernel_v5..v24): v as 2MB pair or
4MB quad DMAs (coarse issues interact badly with the semaphore-pool
pacing), kpool 4-6 / vpool 6-7 rings, weaving v issues between
ring-gated k issues, outs on gpsimd software DGE (50 GB/s -> the drain
waits), outs on the sync queue (2x tried), q in 8 chunks, per-core
rolled batch/i-tile order (HBM skew: no effect), half-tile tail v DMAs,
split last out. Collectives cost a fixed ~85us/NEFF here, ruling out
batch-sharded softmax. Precision floor: fp8 v would add ~2.2% RMS error
(gate headroom is 0.9%) — v must stay bf16, so 21MB on the sync queue
is the byte floor for this sharding. Remaining structure (fast-state
trace): ~7.3us fixed NEFF prologue, first k bytes t=10.2, sync stream
sustains ~380-450 GB/s, MM1 q-gated until ~25 ends ~51.5, MM2 ends
~85 chasing the v tail, out+drain ~6us.
"""
import sys

sys.path.insert(0, "/opt/trn_rl_repo")

import numpy as np
import ml_dtypes

try:
    import types
    import antenv
    import trn_agent_boot.trn_boot as _tb
    if "antenv.axon_hooks" not in sys.modules:
        _hook = _tb._ntff_profile_via_ctypes("/opt/axon/libaxon_pjrt.so")
        _mod = types.ModuleType("antenv.axon_hooks")
        _mod.get_axon_ntff_profile_hook = lambda: _hook
        _mod.set_axon_ntff_profile_hook = lambda h: None
        sys.modules["antenv.axon_hooks"] = _mod
        antenv.axon_hooks = _mod
except Exception:
    pass

import concourse.bass as bass
import concourse.mybir as mybir
import concourse.tile as tile
from concourse import bacc
from concourse.bass_utils import run_bass_kernel_spmd

P = 128
B, C, L = 16, 512, 1024
NCORES = 8
J = L // NCORES   # 128 j-columns per core
BF16 = mybir.dt.bfloat16
F32 = mybir.dt.float32
FP8 = mybir.dt.float8e4
CC_N = C // P     # 4 contraction tiles
IT_N = L // P     # 8 i-tiles
BQ_N = B // 4     # 4 batch-quads

_cached_nc = None


def _build():
    nc = bacc.Bacc("TRN2", target_bir_lowering=False, debug=False,
                   num_devices=NCORES)
    q_ext = nc.dram_tensor("q", [P, B, CC_N, J], BF16, kind="ExternalInput").ap()
    k_ext = nc.dram_tensor("k", [P, IT_N, B, CC_N, P], FP8,
                           kind="ExternalInput").ap()
    v_ext = nc.dram_tensor("v", [P, B, IT_N, C], BF16,
                           kind="ExternalInput").ap()
    out_ext = nc.dram_tensor("out", [P, B, C], BF16,
                             kind="ExternalOutput").ap()

    with tile.TileContext(nc) as tc:
        with (
            tc.tile_pool(name="qpool", bufs=1) as qpool,
            tc.tile_pool(name="kpool", bufs=8) as kpool,
            tc.tile_pool(name="vpool", bufs=10) as vpool,
            tc.tile_pool(name="epool", bufs=1) as epool,
            tc.tile_pool(name="lpool", bufs=1) as lpool,
            tc.tile_pool(name="dpool", bufs=2) as dpool,
            tc.tile_pool(name="spool", bufs=1) as spool,
            tc.tile_pool(name="opool", bufs=2) as opool,
            tc.tile_pool(name="ps1", bufs=4, space="PSUM") as ps1,
            tc.tile_pool(name="ps2", bufs=4, space="PSUM") as ps2,
        ):
            # q on the ACT queue (done ~5us); k then v own the sync queue.
            q_sb = qpool.tile([P, B, CC_N, J], BF16, name="q_all")
            for qs in range(4):
                nc.scalar.dma_start(q_sb[:, qs * 4:(qs + 1) * 4],
                                    q_ext[:, qs * 4:(qs + 1) * 4])

            k_sb = []
            for it in range(IT_N):
                kt = kpool.tile([P, B, CC_N, P], FP8, tag="ktile")
                if it == 0:
                    nc.sync.dma_start(kt[:, 0:4], k_ext[:, it, 0:4])
                    nc.sync.dma_start(kt[:, 4:], k_ext[:, it, 4:])
                elif it < 4:
                    nc.sync.dma_start(kt[:], k_ext[:, it])
                else:
                    nc.scalar.dma_start(kt[:], k_ext[:, it])
                k_sb.append(kt)
            v_sb = []
            for b in range(B):
                vt = vpool.tile([P, IT_N, C], BF16, tag="vtile")
                nc.sync.dma_start(vt[:], v_ext[:, b])
                v_sb.append(vt)

            # e[i_part, it, b, j]: all exp'd scores, bf16, 32KB/partition
            e_st = epool.tile([P, IT_N, B, J], BF16, name="e_st")
            recip_bf = spool.tile([P, IT_N, J], BF16, name="recip_bf")

            # ---- MM1 + exp + d + recip, i-tile major ----
            for it in range(IT_N):
                kt = k_sb[it]
                for bq in range(BQ_N):
                    ps = ps1.tile([P, 4, J], F32, tag="mm1")
                    for s in range(4):
                        b = bq * 4 + s
                        for cc in range(CC_N):
                            nc.tensor.matmul(
                                ps[:, s],
                                kt[:, b, cc, :],
                                q_sb[:, b, cc, :],
                                start=(cc == 0),
                                stop=(cc == CC_N - 1),
                            )
                    nc.scalar.activation(
                        e_st[:, it, bq * 4:(bq + 1) * 4, :], ps[:],
                        mybir.ActivationFunctionType.Exp,
                        scale=float(1.0 / (L ** 0.5)),
                    )
                # d[it] via strided pairwise folds: 16 -> 8 -> 4 -> 2 -> 1
                l8 = lpool.tile([P, 8, J], BF16, tag="l8")
                nc.vector.tensor_add(
                    l8[:], e_st[:, it, 0:16:2, :], e_st[:, it, 1:16:2, :])
                l4 = lpool.tile([P, 4, J], BF16, tag="l4")
                nc.vector.tensor_add(l4[:], l8[:, 0:8:2, :], l8[:, 1:8:2, :])
                l2 = lpool.tile([P, 2, J], BF16, tag="l2")
                nc.vector.tensor_add(l2[:], l4[:, 0:4:2, :], l4[:, 1:4:2, :])
                d32 = dpool.tile([P, J], F32, tag="d32")
                nc.vector.tensor_add(d32[:], l2[:, 0, :], l2[:, 1, :])
                r32 = dpool.tile([P, J], F32, tag="r32")
                nc.vector.reciprocal_approx_fast(r32[:], d32[:])
                nc.vector.tensor_copy(recip_bf[:, it, :], r32[:])
                if it == 3:
                    # first-half probs: runs on VectorE during MM1 of it 4-7
                    for b in range(B):
                        nc.vector.tensor_mul(
                            e_st[:, 0:4, b, :], e_st[:, 0:4, b, :],
                            recip_bf[:, 0:4, :])

            # ---- probs (second half) up front, then MM2 ----
            # Hoisting all prob muls lets MM2 chains run back-to-back on PE
            # instead of waiting cast(b-1) -> prob(b) on VectorE each batch.
            for b in range(B):
                nc.vector.tensor_mul(
                    e_st[:, 4:8, b, :], e_st[:, 4:8, b, :], recip_bf[:, 4:8, :])
            # out DMAs merged 4 batches at a time: out completions recycle
            # into the shared DMA-semaphore pool that also gates the late v
            # dma_start issues on the sync engine; 16 late-completing out
            # DMAs there stall the v stream against MM2 (measured ~5us).
            ot = None
            for b in range(B):
                vt = v_sb[b]
                po = ps2.tile([P, C], F32, tag="mm2")
                for it in range(IT_N):
                    nc.tensor.matmul(
                        po[:],
                        e_st[:, it, b, :],
                        vt[:, it, :],
                        start=(it == 0),
                        stop=(it == IT_N - 1),
                    )
                if b % 4 == 0:
                    ot = opool.tile([P, 4, C], BF16, tag="otile")
                nc.vector.tensor_copy(ot[:, b % 4], po[:])
                if b % 4 == 3:
                    nc.scalar.dma_start(out_ext[:, b - 3:b + 1], ot[:])

    nc.compile()
    return nc


def _prep_inputs(q, k, v):
    """Host-side bf16 cast + DMA-friendly layouts. k_p/v_p shared by cores."""
    q_bf = np.asarray(q).astype(ml_dtypes.bfloat16)
    k_f8 = np.asarray(k).astype(ml_dtypes.float8_e4m3fn)
    v_bf = np.asarray(v).astype(ml_dtypes.bfloat16)

    # k: (B, C, L) -> (c_in 128, it 8, b 16, cc 4, i_in 128)
    k_p = np.ascontiguousarray(
        k_f8.reshape(B, CC_N, P, IT_N, P).transpose(2, 3, 0, 1, 4))
    # v: (B, C, L) -> (i_in 128, b 16, it 8, c 512)
    v_p = np.ascontiguousarray(
        v_bf.reshape(B, C, IT_N, P).transpose(3, 0, 2, 1))
    # q per j-slice: (c_in 128, b 16, cc 4, j 128)
    in_maps = []
    for js in range(NCORES):
        qs = q_bf[:, :, js * J:(js + 1) * J]
        q_p = np.ascontiguousarray(
            qs.reshape(B, CC_N, P, J).transpose(2, 0, 1, 3))
        in_maps.append({"q": q_p, "k": k_p, "v": v_p})
    return in_maps


def kernel(q: np.ndarray, k: np.ndarray, v: np.ndarray) -> np.ndarray:
    """Full inputs (B, C, L) f32 -> full output (B, C, L) f32."""
    global _cached_nc
    assert q.shape == (B, C, L) and k.shape == (B, C, L) and v.shape == (B, C, L)

    in_maps = _prep_inputs(q, k, v)
    if _cached_nc is None:
        _cached_nc = _build()
    res = run_bass_kernel_spmd(_cached_nc, in_maps, list(range(NCORES)))

    # out param: (j_in 128, b 16, c 512); out[b, c, js*128+j_in] = arr[j_in, b, c]
    out = np.concatenate(
        [np.asarray(res.results[core]["out"]).astype(np.float32)
         .transpose(1, 2, 0) for core in range(NCORES)], axis=2)
    return np.ascontiguousarray(out)


if __name__ == "__main__":
    rng = np.random.default_rng(0)
    q = rng.standard_normal((B, C, L)).astype(np.float32)
    k = rng.standard_normal((B, C, L)).astype(np.float32)
    v = rng.standard_normal((B, C, L)).astype(np.float32)
    out = kernel(q=q, k=k, v=v)
    s = np.einsum("bci,bcj->bij", k, q) / np.sqrt(L)
    e = np.exp(s - s.max(axis=0, keepdims=True))
    p = e / e.sum(axis=0, keepdims=True)
    ref = np.einsum("bci,bij->bcj", v, p)
    print("rel fro err:", np.linalg.norm(out - ref) / np.linalg.norm(ref))



# revision 56
# speedup vs baseline: 1.0906x; 1.0906x over previous
"""Trainium2 Bass kernel for nn_Attention_21835613733572 — v9 (j8, P2, fp8 k, deep v + bf16 tree).

reference:
    score = einsum('bci,bcj->bij', k, q) / sqrt(L)       # (B, L, L)
    score = softmax(score, axis=0)                       # over the BATCH axis
    out   = einsum('bci,bij->bcj', v, score)             # (B, C, L)
with B, C, L = 16, 512, 1024 (f32 inputs/outputs).

Distribution: 8 j-slices of 128 columns, one per core; every core holds the
full k and v (the batch-axis softmax needs all 16 batches per (i,j), and
collectives on this fleet cost a fixed ~85us per NEFF — measured — so any
cross-core exchange loses). This is the baseline's sharding, rebuilt around
what the baseline trace showed:

 * P2 layout: MM1 uses stationary k[c,i_tile], moving q[c,j], so scores land
   as e[i_part, j_free] — exactly the lhsT layout MM2 needs. Removes all PE
   transpose matmuls (~14us of PE) the baseline spent.
 * ScalarE runs ONLY Exp (one activation table load, 1.28us per swap
   otherwise): MM1 psums are batch-quad packed ([128,4,128] = one full psum
   bank) so exp is 32 instructions instead of 128+ mixed exp/copy.
 * Denominator d[i,j] = sum_b e[b,i,j] via a 4-level strided pairwise fold
   on VectorE (4 instructions per i-tile, operands [128,8,128]...), not a
   15-add scalar tree.
 * MM2 psum evacuation on VectorE (no scalar table thrash), output DMA on
   the ACT hardware queue (the baseline put it on gpsimd's software DGE,
   which sustained only ~50 GB/s and added a ~30us tail).
 * k then v stream on the sync hardware queue (baseline showed a single
   HWDGE queue sustains ~385 GB/s; total per-core HBM ~420 GB/s).

Per-core input DMA 27.3MB (k fp8 8.4 + q 2.1 + v 16.8). Measured on this
fleet: 96.6-108us HW exec (run-to-run fleet noise is +-5us; interleaved
A/B against the previous best showed the same spread), rel err 1.777e-2
(gate 2e-2; deterministic for the fixed-seed harness inputs).

v20 refinements over v9, from trace forensics:
 * out DMAs merged 4-batches-at-a-time: every dma_start draws a completion
   semaphore from a small pool (~8-10) shared across BOTH HWDGE queues,
   and each issue waits for its semaphore's previous users to COMPLETE.
   With 16 per-batch outs, a late (MM2-gated) out completion gated the v8+
   dma_start issues -> the k/v queue ran dry ~6us at t=60 (visible in the
   v9 trace as a cum-bytes plateau).
 * second-half prob muls hoisted ahead of the MM2 loop: MM2 chains no
   longer serialize on VectorE's cast(b-1) -> prob(b) program order
   (~0.4us/batch).
 * ps1 bufs 3->4 (ps1 4 + ps2 4 = all 8 PSUM banks), vpool 12->13
   (lpool 2->1, dpool 4->2 pay the SBUF).

Experiments that regressed (this session + prior): splitting k or v
across both HWDGE queues in ANY mix (during compute the per-core DMA
total is capped ~340-420 GB/s and concurrent queues just split it; the
critical k stream starves), v as 2MB pair-DMAs (coarser issues interact
badly with the semaphore-pool pacing), kpool 4 or 6 / vpool 6-7 rings,
weaving v issues between ring-gated k issues, outs on gpsimd software
DGE (50 GB/s -> drain waits), outs on the sync queue, per-core rolled
batch/i-tile order (HBM skew: no effect), half-tile tail v DMAs + split
last out. Collectives cost a fixed ~85us/NEFF here, ruling out
batch-sharded softmax. Remaining structure (from the trace): ~7.3us
fixed NEFF prologue before the first dma_start can execute, first k
bytes land ~10.2us, k+v stream on the sync queue sustains ~380-420 GB/s
with MM1 q-gated until ~25us, MM1 ends ~51.5, MM2 is v-arrival-chased
after b11, last matmul ~90.7, out+drain tail ~6us.
"""
import sys

sys.path.insert(0, "/opt/trn_rl_repo")

import numpy as np
import ml_dtypes

try:
    import types
    import antenv
    import trn_agent_boot.trn_boot as _tb
    if "antenv.axon_hooks" not in sys.modules:
        _hook = _tb._ntff_profile_via_ctypes("/opt/axon/libaxon_pjrt.so")
        _mod = types.ModuleType("antenv.axon_hooks")
        _mod.get_axon_ntff_profile_hook = lambda: _hook
        _mod.set_axon_ntff_profile_hook = lambda h: None
        sys.modules["antenv.axon_hooks"] = _mod
        antenv.axon_hooks = _mod
except Exception:
    pass

import concourse.bass as bass
import concourse.mybir as mybir
import concourse.tile as tile
from concourse import bacc
from concourse.bass_utils import run_bass_kernel_spmd

P = 128
B, C, L = 16, 512, 1024
NCORES = 8
J = L // NCORES   # 128 j-columns per core
BF16 = mybir.dt.bfloat16
F32 = mybir.dt.float32
FP8 = mybir.dt.float8e4
CC_N = C // P     # 4 contraction tiles
IT_N = L // P     # 8 i-tiles
BQ_N = B // 4     # 4 batch-quads

_cached_nc = None


def _build():
    nc = bacc.Bacc("TRN2", target_bir_lowering=False, debug=False,
                   num_devices=NCORES)
    q_ext = nc.dram_tensor("q", [P, B, CC_N, J], BF16, kind="ExternalInput").ap()
    k_ext = nc.dram_tensor("k", [P, IT_N, B, CC_N, P], FP8,
                           kind="ExternalInput").ap()
    v_ext = nc.dram_tensor("v", [P, B, IT_N, C], BF16,
                           kind="ExternalInput").ap()
    out_ext = nc.dram_tensor("out", [P, B, C], BF16,
                             kind="ExternalOutput").ap()

    with tile.TileContext(nc) as tc:
        with (
            tc.tile_pool(name="qpool", bufs=1) as qpool,
            tc.tile_pool(name="kpool", bufs=8) as kpool,
            tc.tile_pool(name="vpool", bufs=10) as vpool,
            tc.tile_pool(name="epool", bufs=1) as epool,
            tc.tile_pool(name="lpool", bufs=1) as lpool,
            tc.tile_pool(name="dpool", bufs=2) as dpool,
            tc.tile_pool(name="spool", bufs=1) as spool,
            tc.tile_pool(name="opool", bufs=2) as opool,
            tc.tile_pool(name="ps1", bufs=4, space="PSUM") as ps1,
            tc.tile_pool(name="ps2", bufs=4, space="PSUM") as ps2,
        ):
            # Head swap: k0 rides the scalar queue FIRST (bytes land ~t11,
            # before the sync queue's first bytes), while q rides the sync
            # queue at full rate ahead of k1-3 — MM1 it0 starts ~t12.5
            # with its q quads arriving seamlessly.
            q_sb = qpool.tile([P, B, CC_N, J], BF16, name="q_all")
            for qs in range(4):
                nc.sync.dma_start(q_sb[:, qs * 4:(qs + 1) * 4],
                                  q_ext[:, qs * 4:(qs + 1) * 4])

            k_sb = []
            for it in range(IT_N):
                kt = kpool.tile([P, B, CC_N, P], FP8, tag="ktile")
                if it == 0:
                    nc.scalar.dma_start(kt[:, 0:4], k_ext[:, it, 0:4])
                    nc.scalar.dma_start(kt[:, 4:], k_ext[:, it, 4:])
                elif it < 4:
                    nc.sync.dma_start(kt[:], k_ext[:, it])
                else:
                    nc.scalar.dma_start(kt[:], k_ext[:, it])
                k_sb.append(kt)
            v_sb = []
            for b in range(B):
                vt = vpool.tile([P, IT_N, C], BF16, tag="vtile")
                if b < 14:
                    nc.sync.dma_start(vt[:], v_ext[:, b])
                v_sb.append(vt)

            # e[i_part, it, b, j]: all exp'd scores, bf16, 32KB/partition
            e_st = epool.tile([P, IT_N, B, J], BF16, name="e_st")
            recip_bf = spool.tile([P, IT_N, J], BF16, name="recip_bf")

            # ---- MM1 + exp + d + recip, i-tile major ----
            for it in range(IT_N):
                kt = k_sb[it]
                for bq in range(BQ_N):
                    ps = ps1.tile([P, 4, J], F32, tag="mm1")
                    for s in range(4):
                        b = bq * 4 + s
                        for cc in range(CC_N):
                            nc.tensor.matmul(
                                ps[:, s],
                                kt[:, b, cc, :],
                                q_sb[:, b, cc, :],
                                start=(cc == 0),
                                stop=(cc == CC_N - 1),
                            )
                    nc.scalar.activation(
                        e_st[:, it, bq * 4:(bq + 1) * 4, :], ps[:],
                        mybir.ActivationFunctionType.Exp,
                        scale=float(1.0 / (L ** 0.5)),
                    )
                # d[it] via strided pairwise folds: 16 -> 8 -> 4 -> 2 -> 1
                l8 = lpool.tile([P, 8, J], BF16, tag="l8")
                nc.vector.tensor_add(
                    l8[:], e_st[:, it, 0:16:2, :], e_st[:, it, 1:16:2, :])
                l4 = lpool.tile([P, 4, J], BF16, tag="l4")
                nc.vector.tensor_add(l4[:], l8[:, 0:8:2, :], l8[:, 1:8:2, :])
                l2 = lpool.tile([P, 2, J], BF16, tag="l2")
                nc.vector.tensor_add(l2[:], l4[:, 0:4:2, :], l4[:, 1:4:2, :])
                d32 = dpool.tile([P, J], F32, tag="d32")
                nc.vector.tensor_add(d32[:], l2[:, 0, :], l2[:, 1, :])
                r32 = dpool.tile([P, J], F32, tag="r32")
                nc.vector.reciprocal_approx_fast(r32[:], d32[:])
                nc.vector.tensor_copy(recip_bf[:, it, :], r32[:])
                if it == 3:
                    # first-half probs: runs on VectorE during MM1 of it 4-7
                    for b in range(B):
                        nc.vector.tensor_mul(
                            e_st[:, 0:4, b, :], e_st[:, 0:4, b, :],
                            recip_bf[:, 0:4, :])

            # ---- probs (second half) up front, then MM2 ----
            # Hoisting all prob muls lets MM2 chains run back-to-back on PE
            # instead of waiting cast(b-1) -> prob(b) on VectorE each batch.
            for b in range(B):
                nc.vector.tensor_mul(
                    e_st[:, 4:8, b, :], e_st[:, 4:8, b, :], recip_bf[:, 4:8, :])
            # v14/v15 ride the scalar queue (idle after k4-7 until the
            # outs); sync drops to 18.9MB so v10-13 land by ~t60. Their
            # ring gates (MM2 b4/b5 reads) must clear AFTER the exps, so
            # these dma_starts are emitted here — after all activations
            # in scalar program order — to avoid engine-order deadlock.
            nc.scalar.dma_start(v_sb[14][:], v_ext[:, 14])
            nc.scalar.dma_start(v_sb[15][:], v_ext[:, 15])
            # out DMAs merged 4 batches at a time: out completions recycle
            # into the shared DMA-semaphore pool that also gates the late v
            # dma_start issues on the sync engine; 16 late-completing out
            # DMAs there stall the v stream against MM2 (measured ~5us).
            ot = None
            for b in range(B):
                vt = v_sb[b]
                po = ps2.tile([P, C], F32, tag="mm2")
                for it in range(IT_N):
                    nc.tensor.matmul(
                        po[:],
                        e_st[:, it, b, :],
                        vt[:, it, :],
                        start=(it == 0),
                        stop=(it == IT_N - 1),
                    )
                if b % 4 == 0:
                    ot = opool.tile([P, 4, C], BF16, tag="otile")
                nc.vector.tensor_copy(ot[:, b % 4], po[:])
                if b % 4 == 3:
                    nc.scalar.dma_start(out_ext[:, b - 3:b + 1], ot[:])

    nc.compile()
    return nc


def _prep_inputs(q, k, v):
    """Host-side bf16 cast + DMA-friendly layouts. k_p/v_p shared by cores."""
    q_bf = np.asarray(q).astype(ml_dtypes.bfloat16)
    k_f8 = np.asarray(k).astype(ml_dtypes.float8_e4m3fn)
    v_bf = np.asarray(v).astype(ml_dtypes.bfloat16)

    # k: (B, C, L) -> (c_in 128, it 8, b 16, cc 4, i_in 128)
    k_p = np.ascontiguousarray(
        k_f8.reshape(B, CC_N, P, IT_N, P).transpose(2, 3, 0, 1, 4))
    # v: (B, C, L) -> (i_in 128, b 16, it 8, c 512)
    v_p = np.ascontiguousarray(
        v_bf.reshape(B, C, IT_N, P).transpose(3, 0, 2, 1))
    # q per j-slice: (c_in 128, b 16, cc 4, j 128)
    in_maps = []
    for js in range(NCORES):
        qs = q_bf[:, :, js * J:(js + 1) * J]
        q_p = np.ascontiguousarray(
            qs.reshape(B, CC_N, P, J).transpose(2, 0, 1, 3))
        in_maps.append({"q": q_p, "k": k_p, "v": v_p})
    return in_maps


def kernel(q: np.ndarray, k: np.ndarray, v: np.ndarray) -> np.ndarray:
    """Full inputs (B, C, L) f32 -> full output (B, C, L) f32."""
    global _cached_nc
    assert q.shape == (B, C, L) and k.shape == (B, C, L) and v.shape == (B, C, L)

    in_maps = _prep_inputs(q, k, v)
    if _cached_nc is None:
        _cached_nc = _build()
    res = run_bass_kernel_spmd(_cached_nc, in_maps, list(range(NCORES)))

    # out param: (j_in 128, b 16, c 512); out[b, c, js*128+j_in] = arr[j_in, b, c]
    out = np.concatenate(
        [np.asarray(res.results[core]["out"]).astype(np.float32)
         .transpose(1, 2, 0) for core in range(NCORES)], axis=2)
    return np.ascontiguousarray(out)


if __name__ == "__main__":
    rng = np.random.default_rng(0)
    q = rng.standard_normal((B, C, L)).astype(np.float32)
    k = rng.standard_normal((B, C, L)).astype(np.float32)
    v = rng.standard_normal((B, C, L)).astype(np.float32)
    out = kernel(q=q, k=k, v=v)
    s = np.einsum("bci,bcj->bij", k, q) / np.sqrt(L)
    e = np.exp(s - s.max(axis=0, keepdims=True))
    p = e / e.sum(axis=0, keepdims=True)
    ref = np.einsum("bci,bij->bcj", v, p)
    print("rel fro err:", np.linalg.norm(out - ref) / np.linalg.norm(ref))



# revision 59
# speedup vs baseline: 1.1114x; 1.0190x over previous
"""Trainium2 Bass kernel for nn_Attention_21835613733572 — v9 (j8, P2, fp8 k, deep v + bf16 tree).

reference:
    score = einsum('bci,bcj->bij', k, q) / sqrt(L)       # (B, L, L)
    score = softmax(score, axis=0)                       # over the BATCH axis
    out   = einsum('bci,bij->bcj', v, score)             # (B, C, L)
with B, C, L = 16, 512, 1024 (f32 inputs/outputs).

Distribution: 8 j-slices of 128 columns, one per core; every core holds the
full k and v (the batch-axis softmax needs all 16 batches per (i,j), and
collectives on this fleet cost a fixed ~85us per NEFF — measured — so any
cross-core exchange loses). This is the baseline's sharding, rebuilt around
what the baseline trace showed:

 * P2 layout: MM1 uses stationary k[c,i_tile], moving q[c,j], so scores land
   as e[i_part, j_free] — exactly the lhsT layout MM2 needs. Removes all PE
   transpose matmuls (~14us of PE) the baseline spent.
 * ScalarE runs ONLY Exp (one activation table load, 1.28us per swap
   otherwise): MM1 psums are batch-quad packed ([128,4,128] = one full psum
   bank) so exp is 32 instructions instead of 128+ mixed exp/copy.
 * Denominator d[i,j] = sum_b e[b,i,j] via a 4-level strided pairwise fold
   on VectorE (4 instructions per i-tile, operands [128,8,128]...), not a
   15-add scalar tree.
 * MM2 psum evacuation on VectorE (no scalar table thrash), output DMA on
   the ACT hardware queue (the baseline put it on gpsimd's software DGE,
   which sustained only ~50 GB/s and added a ~30us tail).
 * k then v stream on the sync hardware queue (baseline showed a single
   HWDGE queue sustains ~385 GB/s; total per-core HBM ~420 GB/s).

Per-core input DMA 27.3MB (k fp8 8.4 + q 2.1 + v 16.8). Measured on this
fleet: 96.6-108us HW exec (run-to-run fleet noise is +-5us; interleaved
A/B against the previous best showed the same spread), rel err 1.777e-2
(gate 2e-2; deterministic for the fixed-seed harness inputs).

v20 refinements over v9, from trace forensics:
 * out DMAs merged 4-batches-at-a-time: every dma_start draws a completion
   semaphore from a small pool (~8-10) shared across BOTH HWDGE queues,
   and each issue waits for its semaphore's previous users to COMPLETE.
   With 16 per-batch outs, a late (MM2-gated) out completion gated the v8+
   dma_start issues -> the k/v queue ran dry ~6us at t=60 (visible in the
   v9 trace as a cum-bytes plateau).
 * second-half prob muls hoisted ahead of the MM2 loop: MM2 chains no
   longer serialize on VectorE's cast(b-1) -> prob(b) program order
   (~0.4us/batch).
 * ps1 bufs 3->4 (ps1 4 + ps2 4 = all 8 PSUM banks), vpool 12->13
   (lpool 2->1, dpool 4->2 pay the SBUF).

Experiments that regressed (this session + prior): splitting k or v
across both HWDGE queues in ANY mix (during compute the per-core DMA
total is capped ~340-420 GB/s and concurrent queues just split it; the
critical k stream starves), v as 2MB pair-DMAs (coarser issues interact
badly with the semaphore-pool pacing), kpool 4 or 6 / vpool 6-7 rings,
weaving v issues between ring-gated k issues, outs on gpsimd software
DGE (50 GB/s -> drain waits), outs on the sync queue, per-core rolled
batch/i-tile order (HBM skew: no effect), half-tile tail v DMAs + split
last out. Collectives cost a fixed ~85us/NEFF here, ruling out
batch-sharded softmax. Remaining structure (from the trace): ~7.3us
fixed NEFF prologue before the first dma_start can execute, first k
bytes land ~10.2us, k+v stream on the sync queue sustains ~380-420 GB/s
with MM1 q-gated until ~25us, MM1 ends ~51.5, MM2 is v-arrival-chased
after b11, last matmul ~90.7, out+drain tail ~6us.
"""
import sys

sys.path.insert(0, "/opt/trn_rl_repo")

import numpy as np
import ml_dtypes

try:
    import types
    import antenv
    import trn_agent_boot.trn_boot as _tb
    if "antenv.axon_hooks" not in sys.modules:
        _hook = _tb._ntff_profile_via_ctypes("/opt/axon/libaxon_pjrt.so")
        _mod = types.ModuleType("antenv.axon_hooks")
        _mod.get_axon_ntff_profile_hook = lambda: _hook
        _mod.set_axon_ntff_profile_hook = lambda h: None
        sys.modules["antenv.axon_hooks"] = _mod
        antenv.axon_hooks = _mod
except Exception:
    pass

import concourse.bass as bass
import concourse.mybir as mybir
import concourse.tile as tile
from concourse import bacc
from concourse.bass_utils import run_bass_kernel_spmd

P = 128
B, C, L = 16, 512, 1024
NCORES = 8
J = L // NCORES   # 128 j-columns per core
BF16 = mybir.dt.bfloat16
F32 = mybir.dt.float32
FP8 = mybir.dt.float8e4
CC_N = C // P     # 4 contraction tiles
IT_N = L // P     # 8 i-tiles
BQ_N = B // 4     # 4 batch-quads

_cached_nc = None


def _build():
    nc = bacc.Bacc("TRN2", target_bir_lowering=False, debug=False,
                   num_devices=NCORES)
    q_ext = nc.dram_tensor("q", [P, B, CC_N, J], BF16, kind="ExternalInput").ap()
    k_ext = nc.dram_tensor("k", [P, IT_N, B, CC_N, P], FP8,
                           kind="ExternalInput").ap()
    v_ext = nc.dram_tensor("v", [P, B, IT_N, C], BF16,
                           kind="ExternalInput").ap()
    out_ext = nc.dram_tensor("out", [P, B, C], BF16,
                             kind="ExternalOutput").ap()

    with tile.TileContext(nc) as tc:
        with (
            tc.tile_pool(name="qpool", bufs=1) as qpool,
            tc.tile_pool(name="kpool", bufs=8) as kpool,
            tc.tile_pool(name="vpool", bufs=10) as vpool,
            tc.tile_pool(name="epool", bufs=1) as epool,
            tc.tile_pool(name="lpool", bufs=1) as lpool,
            tc.tile_pool(name="dpool", bufs=2) as dpool,
            tc.tile_pool(name="spool", bufs=1) as spool,
            tc.tile_pool(name="opool", bufs=2) as opool,
            tc.tile_pool(name="ps1", bufs=4, space="PSUM") as ps1,
            tc.tile_pool(name="ps2", bufs=4, space="PSUM") as ps2,
        ):
            # Head swap: k0 rides the scalar queue FIRST (bytes land ~t11,
            # before the sync queue's first bytes), while q rides the sync
            # queue at full rate ahead of k1-3 — MM1 it0 starts ~t12.5
            # with its q quads arriving seamlessly.
            q_sb = qpool.tile([P, B, CC_N, J], BF16, name="q_all")
            for qs in range(4):
                nc.sync.dma_start(q_sb[:, qs * 4:(qs + 1) * 4],
                                  q_ext[:, qs * 4:(qs + 1) * 4])

            k_sb = []
            for it in range(IT_N):
                kt = kpool.tile([P, B, CC_N, P], FP8, tag="ktile")
                if it == 0:
                    nc.scalar.dma_start(kt[:, 0:4], k_ext[:, it, 0:4])
                    nc.scalar.dma_start(kt[:, 4:], k_ext[:, it, 4:])
                elif it < 6:
                    nc.sync.dma_start(kt[:], k_ext[:, it])
                else:
                    nc.scalar.dma_start(kt[:], k_ext[:, it])
                k_sb.append(kt)
            v_sb = []
            for b in range(B):
                vt = vpool.tile([P, IT_N, C], BF16, tag="vtile")
                if b < 14:
                    nc.sync.dma_start(vt[:], v_ext[:, b])
                v_sb.append(vt)

            # e[i_part, it, b, j]: all exp'd scores, bf16, 32KB/partition
            e_st = epool.tile([P, IT_N, B, J], BF16, name="e_st")
            recip_bf = spool.tile([P, IT_N, J], BF16, name="recip_bf")

            # ---- MM1 + exp + d + recip, i-tile major ----
            for it in range(IT_N):
                kt = k_sb[it]
                for bq in range(BQ_N):
                    ps = ps1.tile([P, 4, J], F32, tag="mm1")
                    for s in range(4):
                        b = bq * 4 + s
                        for cc in range(CC_N):
                            nc.tensor.matmul(
                                ps[:, s],
                                kt[:, b, cc, :],
                                q_sb[:, b, cc, :],
                                start=(cc == 0),
                                stop=(cc == CC_N - 1),
                            )
                    nc.scalar.activation(
                        e_st[:, it, bq * 4:(bq + 1) * 4, :], ps[:],
                        mybir.ActivationFunctionType.Exp,
                        scale=float(1.0 / (L ** 0.5)),
                    )
                # d[it] via strided pairwise folds: 16 -> 8 -> 4 -> 2 -> 1
                l8 = lpool.tile([P, 8, J], BF16, tag="l8")
                nc.vector.tensor_add(
                    l8[:], e_st[:, it, 0:16:2, :], e_st[:, it, 1:16:2, :])
                l4 = lpool.tile([P, 4, J], BF16, tag="l4")
                nc.vector.tensor_add(l4[:], l8[:, 0:8:2, :], l8[:, 1:8:2, :])
                l2 = lpool.tile([P, 2, J], BF16, tag="l2")
                nc.vector.tensor_add(l2[:], l4[:, 0:4:2, :], l4[:, 1:4:2, :])
                d32 = dpool.tile([P, J], F32, tag="d32")
                nc.vector.tensor_add(d32[:], l2[:, 0, :], l2[:, 1, :])
                r32 = dpool.tile([P, J], F32, tag="r32")
                nc.vector.reciprocal_approx_fast(r32[:], d32[:])
                nc.vector.tensor_copy(recip_bf[:, it, :], r32[:])
                if it == 3:
                    # first-half probs: runs on VectorE during MM1 of it 4-7
                    for b in range(B):
                        nc.vector.tensor_mul(
                            e_st[:, 0:4, b, :], e_st[:, 0:4, b, :],
                            recip_bf[:, 0:4, :])

            # ---- probs (second half) up front, then MM2 ----
            # Hoisting all prob muls lets MM2 chains run back-to-back on PE
            # instead of waiting cast(b-1) -> prob(b) on VectorE each batch.
            for b in range(B):
                nc.vector.tensor_mul(
                    e_st[:, 4:8, b, :], e_st[:, 4:8, b, :], recip_bf[:, 4:8, :])
            # v14/v15 ride the scalar queue (idle after k4-7 until the
            # outs); sync drops to 18.9MB so v10-13 land by ~t60. Their
            # ring gates (MM2 b4/b5 reads) must clear AFTER the exps, so
            # these dma_starts are emitted here — after all activations
            # in scalar program order — to avoid engine-order deadlock.
            nc.scalar.dma_start(v_sb[14][:], v_ext[:, 14])
            nc.scalar.dma_start(v_sb[15][:], v_ext[:, 15])
            # out DMAs merged 4 batches at a time: out completions recycle
            # into the shared DMA-semaphore pool that also gates the late v
            # dma_start issues on the sync engine; 16 late-completing out
            # DMAs there stall the v stream against MM2 (measured ~5us).
            ot = None
            for b in range(B):
                vt = v_sb[b]
                po = ps2.tile([P, C], F32, tag="mm2")
                for it in range(IT_N):
                    nc.tensor.matmul(
                        po[:],
                        e_st[:, it, b, :],
                        vt[:, it, :],
                        start=(it == 0),
                        stop=(it == IT_N - 1),
                    )
                if b % 4 == 0:
                    ot = opool.tile([P, 4, C], BF16, tag="otile")
                nc.vector.tensor_copy(ot[:, b % 4], po[:])
                if b % 4 == 3:
                    nc.scalar.dma_start(out_ext[:, b - 3:b + 1], ot[:])

    nc.compile()
    return nc


def _prep_inputs(q, k, v):
    """Host-side bf16 cast + DMA-friendly layouts. k_p/v_p shared by cores."""
    q_bf = np.asarray(q).astype(ml_dtypes.bfloat16)
    k_f8 = np.asarray(k).astype(ml_dtypes.float8_e4m3fn)
    v_bf = np.asarray(v).astype(ml_dtypes.bfloat16)

    # k: (B, C, L) -> (c_in 128, it 8, b 16, cc 4, i_in 128)
    k_p = np.ascontiguousarray(
        k_f8.reshape(B, CC_N, P, IT_N, P).transpose(2, 3, 0, 1, 4))
    # v: (B, C, L) -> (i_in 128, b 16, it 8, c 512)
    v_p = np.ascontiguousarray(
        v_bf.reshape(B, C, IT_N, P).transpose(3, 0, 2, 1))
    # q per j-slice: (c_in 128, b 16, cc 4, j 128)
    in_maps = []
    for js in range(NCORES):
        qs = q_bf[:, :, js * J:(js + 1) * J]
        q_p = np.ascontiguousarray(
            qs.reshape(B, CC_N, P, J).transpose(2, 0, 1, 3))
        in_maps.append({"q": q_p, "k": k_p, "v": v_p})
    return in_maps


def kernel(q: np.ndarray, k: np.ndarray, v: np.ndarray) -> np.ndarray:
    """Full inputs (B, C, L) f32 -> full output (B, C, L) f32."""
    global _cached_nc
    assert q.shape == (B, C, L) and k.shape == (B, C, L) and v.shape == (B, C, L)

    in_maps = _prep_inputs(q, k, v)
    if _cached_nc is None:
        _cached_nc = _build()
    res = run_bass_kernel_spmd(_cached_nc, in_maps, list(range(NCORES)))

    # out param: (j_in 128, b 16, c 512); out[b, c, js*128+j_in] = arr[j_in, b, c]
    out = np.concatenate(
        [np.asarray(res.results[core]["out"]).astype(np.float32)
         .transpose(1, 2, 0) for core in range(NCORES)], axis=2)
    return np.ascontiguousarray(out)


if __name__ == "__main__":
    rng = np.random.default_rng(0)
    q = rng.standard_normal((B, C, L)).astype(np.float32)
    k = rng.standard_normal((B, C, L)).astype(np.float32)
    v = rng.standard_normal((B, C, L)).astype(np.float32)
    out = kernel(q=q, k=k, v=v)
    s = np.einsum("bci,bcj->bij", k, q) / np.sqrt(L)
    e = np.exp(s - s.max(axis=0, keepdims=True))
    p = e / e.sum(axis=0, keepdims=True)
    ref = np.einsum("bci,bij->bcj", v, p)
    print("rel fro err:", np.linalg.norm(out - ref) / np.linalg.norm(ref))

